# revision 39
# baseline (speedup 1.0000x reference)
"""Trainium2 Bass kernel for GroupedQueryAttention (anti-causal mask variant).

Reference semantics (B=2, S=2048, D=4096, 32 Q heads, 4 KV heads, dk=128):
  Q = x@Wq, K = x@Wk, V = x@Wv (heads split), GQA repeat KV x8.
  scores = Q K^T / sqrt(dk); mask = triu(ones, k=1); scores = where(mask==0, -1e9, scores)
    -> keeps STRICT UPPER triangle (k > q, anti-causal). Rows with no valid key
       (q == S-1) become a uniform softmax over all S keys.
  out = softmax(scores) @ V; out = out @ Wo.

Sharding: 8 cores, 4 Q heads + their 1 shared KV head per core. Each core
computes a partial out = attn_heads @ Wo_rows_slice; host sums the 8 partials.

Per-core design (bf16 matmul inputs, fp32 PSUM accumulation):
  - x^T chunks by PE transposes; Q^T/K^T/V^T projections in [dk, seq] layout.
    Projection matmuls lag the transposes by one chunk group (software
    pipeline) so the PSUM->SBUF copy of x^T never stalls the PE.
  - scores TRANSPOSED: sT[k, q] = K^T chunk x Q^T; exp on Act engine; the
    anti-causal mask is applied AFTER exp by affine_select (fill 0, or
    exp(-30) for the last q block whose fully-masked rows must come out
    uniform). Attention is one flat software-pipelined stream over
    (head, q-block, k-chunk) with scores running 3 chunks ahead of AV.
  - softmax denominator: GpSimd accumulates exp chunks into racc (fp32);
    one float32r ones-matmul per (head, q-block) broadcasts column sums to
    PSUM; DVE reciprocal + multiply normalize into bf16 att tiles.
  - skipped fully-masked chunks of the last q block are added analytically:
    r += n_skip*128*exp(-30), out^T += exp(-30)*cumsum(V) (per batch).
  - output projection: att^T chunks (lhsT) x Wo tiles, partials staged bf16.
"""

import sys
from contextlib import ExitStack

import numpy as np

for _p in ("/opt/trn_rl_repo",):
    if _p not in sys.path:
        sys.path.insert(0, _p)

import bass_rust
import concourse.bass as bass
import concourse.mybir as mybir
import concourse.tile as tile
from concourse.masks import make_identity


def _split_multiwaits(nc):
    """This walrus build encodes at most ONE sem wait per instruction.
    Tile's wait-assignment can attach several; hoist the extras onto fresh
    single-wait NoOps emitted immediately before the instruction on the same
    engine stream. Tile emits instructions in schedule order, so every wait's
    producer precedes the waiting instruction in-stream and the stall cannot
    deadlock."""
    for fn in nc.m.functions:
        for blk in fn.blocks:
            newlist = []
            for ins in blk.instructions:
                si = ins.sync_info
                n = len(si.on_wait) if si is not None else 0
                if n > 1:
                    waits = list(si.on_wait)
                    for j, w in enumerate(waits[:-1]):
                        nop = mybir.InstNoOp(
                            name=f"{ins.name}-hw{j}", engine=ins.engine,
                            ins=[], outs=[],
                            sync_info=bass_rust.SyncInfo(on_wait=[w],
                                                         on_update=[]))
                        nc.register_instruction(nop, overwrite=True)
                        newlist.append(nop)
                    si.on_wait = waits[-1:]
                newlist.append(ins)
            blk.instructions = newlist

B, S, D = 2, 2048, 4096
NQ, NKV, DK = 32, 4, 128
NCORES = 8
HPC = NQ // NCORES          # 4 q heads per core
DKC = HPC * DK              # 512 proj cols per core
SCALE = 1.0 / float(np.sqrt(DK))
MV = 30.0                   # masked logit magnitude (post-scale)
EXP_M = float(np.exp(-MV))
QB = 512                    # q block (matmul moving free dim)
KC = 128                    # k chunk (PE contraction/partition dim)
F32 = mybir.dt.float32
F32R = mybir.dt.float32r
BF16 = mybir.dt.bfloat16
EXP = mybir.ActivationFunctionType.Exp
GT = mybir.AluOpType.is_gt


def build_program(s=S):
    """Build the per-core Bass/Tile program. Same program for all 8 cores
    (SPMD); per-core weight slices are supplied via the input maps."""
    nqb = s // QB            # q blocks
    nkc = s // KC            # k chunks
    nd = D // KC             # D contraction chunks (32)
    ndq = 4                  # x loaded in 4 column quarters
    dq = D // ndq            # 1024
    nskip = 4 * (nqb - 1)    # fully-masked k chunks of the last q block

    nc = bass.Bass("TRN2", target_bir_lowering=False, debug=False,
                   num_devices=NCORES)
    x = nc.dram_tensor("x", [B, s, D], BF16, kind="ExternalInput").ap()
    wq = nc.dram_tensor("wq", [D, DKC], BF16, kind="ExternalInput").ap()
    wk = nc.dram_tensor("wk", [D, DK], BF16, kind="ExternalInput").ap()
    wv = nc.dram_tensor("wv", [D, DK], BF16, kind="ExternalInput").ap()
    wo = nc.dram_tensor("wo", [DKC, D], BF16, kind="ExternalInput").ap()
    out = nc.dram_tensor("out", [B, s, D], BF16, kind="ExternalOutput").ap()

    xf = x.rearrange("b s d -> (b s) d")
    of = out.rearrange("b s d -> (b s) d")

    with tile.TileContext(nc) as tc, ExitStack() as ctx:
        consts = ctx.enter_context(tc.tile_pool(name="consts", bufs=1))
        ident = consts.tile([128, 128], BF16, name="ident", tag="ident")
        make_identity(nc, ident)
        ones = consts.tile([128, 128], BF16, name="ones", tag="ones")
        nc.vector.memset(ones, 1.0)
        # ---------- weights: loaded once, resident for both batches ----------
        # Weight DMAs go on the gpsimd DGE queue so they flow in parallel
        # with the x transpose DMAs on the sync queue.
        wpool = ctx.enter_context(tc.tile_pool(name="wqkv", bufs=1))
        wk_t = wpool.tile([128, nd, DK], BF16, name="wk_t", tag="wk_t")
        wkr = wk.rearrange("(c p) n -> p c n", p=128)
        wv_t = wpool.tile([128, nd, DK], BF16, name="wv_t", tag="wv_t")
        wvr = wv.rearrange("(c p) n -> p c n", p=128)
        wq_t = wpool.tile([128, nd, DKC], BF16, name="wq_t", tag="wq_t")
        wqr = wq.rearrange("(c p) n -> p c n", p=128)
        nnb = D // QB     # 8 column blocks of Wo
        wo_t = wpool.tile([128, HPC, nnb, QB], BF16, name="wo_t", tag="wo_t")
        c8 = nd // 4

        def emit_w_dmas_early():
            # first quarter of wk/wv/wq: everything qb0's first chunks need
            nc.gpsimd.dma_start(out=wk_t[:, 0:c8, :], in_=wkr[:, 0:c8, :])
            nc.gpsimd.dma_start(out=wv_t[:, 0:c8, :], in_=wvr[:, 0:c8, :])
            nc.gpsimd.dma_start(out=wq_t[:, 0:c8, :], in_=wqr[:, 0:c8, :])

        def emit_w_dmas_rest():
            nc.gpsimd.dma_start(out=wk_t[:, c8:, :], in_=wkr[:, c8:, :])
            nc.gpsimd.dma_start(out=wv_t[:, c8:, :], in_=wvr[:, c8:, :])
            for i in range(1, 4):
                csl = slice(i * c8, (i + 1) * c8)
                nc.gpsimd.dma_start(out=wq_t[:, csl, :], in_=wqr[:, csl, :])

        def emit_wo_dma():
            # 4MB load only needed by the output projection; keep it out of
            # the early DMA chain
            nc.gpsimd.dma_start(
                out=wo_t,
                in_=wo.rearrange("(c p) (nb n) -> p c nb n", p=128, n=QB))

        # x^T transpose-DMA groups; pool is global so batch b+1's first
        # groups can prefetch during batch b's output projection.
        xtg = ctx.enter_context(tc.tile_pool(name="xtg", bufs=4))
        xt_pref = {}

        def emit_xt_group(bb, qb, g):
            xT = xtg.tile([128, nd // ndq, QB], BF16, name="xtg", tag="xtg")
            row0 = bb * s + qb * QB
            nc.sync.dma_start_transpose(
                xT, xf[row0:row0 + QB, g * dq:(g + 1) * dq])
            return xT

        for b in range(B):
            with ExitStack() as bctx:
                bpool = bctx.enter_context(tc.tile_pool(name=f"bp{b}", bufs=1))
                qt = [bpool.tile([128, s], BF16, name=f"qt{b}_{h}", tag=f"qt{h}")
                      for h in range(HPC)]
                kt = bpool.tile([128, s], BF16, name=f"kt{b}", tag="kt")
                vt = bpool.tile([128, s], BF16, name=f"vt{b}", tag="vt")
                vn = bpool.tile([128, s], BF16, name=f"vn{b}", tag="vn")

                # ---------- projection phase: Q^T, K^T, V^T ----------
                # x^T chunks arrive pre-transposed straight from DRAM via the
                # DMA XBAR (2-byte transpose mode): no PE transposes, no
                # PSUM->SBUF staging copies.
                with ExitStack() as pctx:
                    ppool = pctx.enter_context(
                        tc.tile_pool(name="projpsum", bufs=1, space="PSUM"))

                    for qb in range(nqb):
                        pq = [ppool.tile([128, QB], F32, name=f"pq{h}", tag=f"pq{h}")
                              for h in range(HPC)]
                        pk = ppool.tile([128, QB], F32, name="pk", tag="pk")
                        pv = ppool.tile([128, QB], F32, name="pv", tag="pv")

                        def emit_mms(xT, kcg):
                            st = kcg == 0
                            sp = kcg == nd - 1
                            for h in range(HPC):
                                nc.tensor.matmul(
                                    pq[h], wq_t[:, kcg, h * 128:(h + 1) * 128],
                                    xT, start=st, stop=sp)
                            nc.tensor.matmul(pk, wk_t[:, kcg, :], xT,
                                             start=st, stop=sp)
                            nc.tensor.matmul(pv, wv_t[:, kcg, :], xT,
                                             start=st, stop=sp)

                        if b == 0 and qb == 0:
                            # hybrid startup: plain x row loads + PE transposes
                            # keep the PE busy while the serialized DMA chain
                            # (weights + later transpose-DMAs) warms up
                            emit_w_dmas_early()
                            with ExitStack() as sctx:
                                x0p = sctx.enter_context(
                                    tc.tile_pool(name="x0load", bufs=4))
                                x0t = sctx.enter_context(
                                    tc.tile_pool(name="x0tsb", bufs=3))
                                t0p = sctx.enter_context(
                                    tc.tile_pool(name="tr0psum", bufs=2,
                                                 space="PSUM"))
                                halves = []
                                for dh in range(2):
                                    xts = []
                                    for rt in range(4):
                                        xt_ = x0p.tile([128, D // 2], BF16,
                                                       name="x0", tag="x0")
                                        row0 = b * s + rt * 128
                                        nc.sync.dma_start(
                                            out=xt_,
                                            in_=xf[row0:row0 + 128,
                                                   dh * (D // 2):(dh + 1) * (D // 2)])
                                        xts.append(xt_)
                                    halves.append(xts)
                                    if dh == 0:
                                        emit_w_dmas_rest()
                                pend = None
                                for kcg in range(nd):
                                    xts = halves[kcg // (nd // 2)]
                                    kcl = kcg % (nd // 2)
                                    ptp = t0p.tile([128, QB], BF16, name="ptp",
                                                   tag="ptp")
                                    for rt in range(4):
                                        nc.tensor.transpose(
                                            ptp[:, rt * 128:(rt + 1) * 128],
                                            xts[rt][:, kcl * 128:(kcl + 1) * 128],
                                            ident)
                                    xT = x0t.tile([128, QB], BF16, name="xT0",
                                                  tag="xT0")
                                    if kcg % 2 == 0:
                                        nc.vector.tensor_copy(xT, ptp)
                                    else:
                                        nc.scalar.copy(xT, ptp)
                                    if pend is not None:
                                        emit_mms(*pend)
                                    pend = (xT, kcg)
                                emit_mms(*pend)
                        else:
                            gts = []
                            for g in range(ndq):
                                if (b, qb, g) in xt_pref:
                                    gts.append(xt_pref.pop((b, qb, g)))
                                    continue
                                gts.append(emit_xt_group(b, qb, g))
                            for kcg in range(nd):
                                emit_mms(
                                    gts[kcg // (nd // ndq)][:, kcg % (nd // ndq), :],
                                    kcg)
                        sl = slice(qb * QB, (qb + 1) * QB)
                        for h in range(HPC):
                            if h % 2 == 0:
                                nc.vector.tensor_copy(qt[h][:, sl], pq[h])
                            else:
                                nc.scalar.copy(qt[h][:, sl], pq[h])
                        nc.vector.tensor_copy(kt[:, sl], pk)
                        nc.scalar.copy(vt[:, sl], pv)

                # ---------- V^T -> V natural ----------
                with ExitStack() as vctx:
                    vpsum = vctx.enter_context(
                        tc.tile_pool(name="vtpsum", bufs=2, space="PSUM"))
                    for kc in range(nkc):
                        pvt = vpsum.tile([128, 128], BF16, name="pvt", tag="pvt")
                        nc.tensor.transpose(
                            pvt, vt[:, kc * 128:(kc + 1) * 128], ident)
                        nc.any.tensor_copy(vn[:, kc * 128:(kc + 1) * 128], pvt)

                # ---------- attention ----------
                apool = bctx.enter_context(tc.tile_pool(name=f"att{b}", bufs=1))
                att = [apool.tile([128, s], BF16, name=f"att{b}_{h}", tag=f"att{h}")
                       for h in range(HPC)]
                with ExitStack() as actx:
                    if b == 0:
                        emit_wo_dma()
                    cvpool = actx.enter_context(tc.tile_pool(name="cvsb", bufs=1))
                    cv = cvpool.tile([128, 1], F32, name="cv", tag="cv")
                    # cumsum-V correction for the last q block (head-indep.)
                    with tc.tile_pool(name="cvps", bufs=1, space="PSUM") as cps:
                        pc = cps.tile([128, 1], F32, name="pc", tag="pc")
                        for i in range(nskip):
                            nc.tensor.matmul(
                                pc, vn[:, i * 128:(i + 1) * 128], ones[:, 0:1],
                                start=(i == 0), stop=(i == nskip - 1))
                        nc.scalar.mul(cv, pc, EXP_M)

                    spool = actx.enter_context(tc.tile_pool(name="attsb", bufs=3))
                    n2pool = actx.enter_context(tc.tile_pool(name="attn2", bufs=2))
                    aps4 = actx.enter_context(
                        tc.tile_pool(name="atps4", bufs=3, space="PSUM"))
                    aps2 = actx.enter_context(
                        tc.tile_pool(name="atps2", bufs=1, space="PSUM"))

                    # flat PAIRED chunk stream: two k-chunks share one PSUM
                    # scores tile and a single exp; diag chunks last within
                    # each q block, width-trimmed to the valid q prefix
                    # (except the last q block which needs exp(-30) fills).
                    PAIRS = []
                    for h in range(HPC):
                        for qb in range(nqb):
                            base = 4 * qb
                            lastqb = qb == nqb - 1
                            nond = list(range(base + 4, nkc))
                            if not lastqb:
                                seq = nond + [base + 3, base + 2, base + 1, base]
                                ws = [QB] * len(nond) + [512, 384, 256, 128]
                            else:
                                seq = [base + 3, base + 2, base + 1, base]
                                ws = [QB] * 4
                            halves = []
                            for kc, w in zip(seq, ws):
                                halves.append(dict(kc=kc, d=kc - base, w=w))
                            for i in range(0, len(halves), 2):
                                h0, h1 = halves[i], halves[i + 1]
                                h0['off'] = 0
                                h1['off'] = h0['w']
                                PAIRS.append(dict(
                                    h=h, qb=qb, lastqb=lastqb, hv=[h0, h1],
                                    first=(i == 0),
                                    last=(i + 2 >= len(halves))))
                    LAP = 2      # pair lookahead depth
                    state = {}

                    def emit_sc(p):
                        ps = aps4.tile([128, 2 * QB], F32, name="ps", tag="ps")
                        q0 = p['qb'] * QB
                        for hv in p['hv']:
                            o, w, kc = hv['off'], hv['w'], hv['kc']
                            nc.tensor.matmul(
                                ps[:, o:o + w], kt[:, kc * 128:(kc + 1) * 128],
                                qt[p['h']][:, q0:q0 + w], start=True, stop=True)
                        p['ps'] = ps

                    def emit_rest(p):
                        h, qb = p['h'], p['qb']
                        qsl = slice(qb * QB, (qb + 1) * QB)
                        wtot = p['hv'][1]['off'] + p['hv'][1]['w']
                        pt = spool.tile([128, 2 * QB], BF16, name="pt", tag="pt")
                        nc.scalar.activation(pt[:, 0:wtot], p['ps'][:, 0:wtot],
                                             EXP, scale=SCALE)
                        for hv in p['hv']:
                            o, w, d = hv['off'], hv['w'], hv['d']
                            if d >= 4:
                                continue
                            if p['lastqb']:
                                # keep where k > q: r + 128*d - c > 0
                                nc.gpsimd.affine_select(
                                    out=pt[:, o:o + w], in_=pt[:, o:o + w],
                                    compare_op=GT, fill=EXP_M,
                                    base=128 * d, pattern=[[-1, w]],
                                    channel_multiplier=1)
                            else:
                                # only the boundary 128-col block is mixed
                                bs = o + w - 128
                                nc.gpsimd.affine_select(
                                    out=pt[:, bs:bs + 128], in_=pt[:, bs:bs + 128],
                                    compare_op=GT, fill=0.0,
                                    base=0, pattern=[[-1, 128]],
                                    channel_multiplier=1)
                        if p['first']:
                            po = aps2.tile([128, QB], F32, name="po", tag="po")
                            pr = aps2.tile([128, QB], F32, name="pr", tag="pr")
                            state['po'], state['pr'] = po, pr
                        else:
                            po, pr = state['po'], state['pr']
                        for i, hv in enumerate(p['hv']):
                            o, w, kc = hv['off'], hv['w'], hv['kc']
                            st = p['first'] and i == 0
                            sp = p['last'] and i == 1
                            nc.tensor.matmul(
                                po[:, 0:w], vn[:, kc * 128:(kc + 1) * 128],
                                pt[:, o:o + w], start=st, stop=sp)
                            nc.tensor.matmul(
                                pr[:, 0:w], ones, pt[:, o:o + w],
                                start=st, stop=sp)
                        if not p['last']:
                            return
                        # ---- group finalize: drain PSUM fast, then normalize
                        pos = n2pool.tile([128, QB], BF16, name="pos", tag="pos")
                        nc.scalar.copy(pos, po)
                        prs = n2pool.tile([128, QB], F32, name="prs", tag="prs")
                        nc.scalar.copy(prs, pr)
                        rr = n2pool.tile([128, QB], F32, name="rr", tag="rr")
                        if p['lastqb']:
                            nc.vector.tensor_scalar_add(
                                prs, prs, float(nskip * 128 * EXP_M))
                            nc.vector.reciprocal(rr, prs)
                            tno = n2pool.tile([128, QB], F32, name="tno",
                                              tag="tno")
                            nc.vector.tensor_scalar_add(tno, pos, cv)
                            nc.vector.tensor_mul(att[h][:, qsl], tno, rr)
                        else:
                            nc.vector.reciprocal(rr, prs)
                            nc.vector.tensor_mul(att[h][:, qsl], pos, rr)

                    for j in range(min(LAP, len(PAIRS))):
                        emit_sc(PAIRS[j])
                    for j in range(len(PAIRS)):
                        if j + LAP < len(PAIRS):
                            emit_sc(PAIRS[j + LAP])
                        emit_rest(PAIRS[j])

                # ---------- output projection (partial: this core's heads) ----
                with ExitStack() as wctx:
                    opsum = wctx.enter_context(
                        tc.tile_pool(name="opsum", bufs=4, space="PSUM"))
                    stpool = wctx.enter_context(tc.tile_pool(name="ostage", bufs=2))
                    for qti in range(s // 128):
                        stg = stpool.tile([128, D], BF16, name="stg", tag="stg")
                        for nb in range(nnb):
                            po2 = opsum.tile([128, QB], F32, name="po2", tag="po2")
                            for c in range(HPC):
                                nc.tensor.matmul(
                                    po2, att[c][:, qti * 128:(qti + 1) * 128],
                                    wo_t[:, c, nb, :],
                                    start=(c == 0), stop=(c == HPC - 1))
                            if nb % 2 == 0:
                                nc.scalar.copy(stg[:, nb * QB:(nb + 1) * QB], po2)
                            else:
                                nc.vector.tensor_copy(
                                    stg[:, nb * QB:(nb + 1) * QB], po2)
                        row0 = b * s + qti * 128
                        nc.sync.dma_start(out=of[row0:row0 + 128, :], in_=stg)
    _split_multiwaits(nc)
    return nc


_PROG = {}


def _get_program(s=S):
    if s not in _PROG:
        _PROG[s] = build_program(s)
    return _PROG[s]


def _np_bf16():
    return mybir.dt.np(BF16)


_XCAST = {}


def core_in_map(c, x, Wq, Wk, Wv, Wo):
    bf = _np_bf16()
    h0 = c * HPC
    kv = (c * HPC) // (NQ // NKV)
    # cast x once, share across the 8 per-core input maps
    key = id(x)
    if key not in _XCAST:
        _XCAST.clear()
        _XCAST[key] = np.ascontiguousarray(np.asarray(x, np.float32)).astype(bf)
    return {
        "x": _XCAST[key],
        "wq": np.ascontiguousarray(
            np.asarray(Wq, np.float32)[:, h0 * DK:(h0 + HPC) * DK]).astype(bf),
        "wk": np.ascontiguousarray(
            np.asarray(Wk, np.float32)[:, kv * DK:(kv + 1) * DK]).astype(bf),
        "wv": np.ascontiguousarray(
            np.asarray(Wv, np.float32)[:, kv * DK:(kv + 1) * DK]).astype(bf),
        "wo": np.ascontiguousarray(
            np.asarray(Wo, np.float32)[h0 * DK:(h0 + HPC) * DK, :]).astype(bf),
    }


def kernel(x, Wq, Wk, Wv, Wo, **kw):
    from concourse.bass_utils import run_bass_kernel_spmd

    nc = _get_program(np.asarray(x).shape[1])
    in_maps = [core_in_map(c, x, Wq, Wk, Wv, Wo) for c in range(NCORES)]
    res = run_bass_kernel_spmd(nc, in_maps, core_ids=list(range(NCORES)), **kw)
    acc = np.zeros(np.asarray(x).shape, np.float64)
    for r in res.results:
        acc += np.asarray(r["out"], np.float32)
    return acc.astype(np.float32)
